# revision 5
# baseline (speedup 1.0000x reference)
"""CAM graph layer (message passing + Linear + ELU) on 8 Trainium2 NeuronCores.

Reference computation (per batch b of N=21 joints, F=256 features):
    x_agg[b,i] = sum_j cam[i,j] * x[b,j]            (21x21 aggregation)
    y = ELU(concat([x_agg, x], -1) @ W.T + b)       (Linear 512->256)

Kernel restructure:
    P  = x @ [W1.T | W2.T]   (one 512-wide matmul; W1/W2 = halves of W)
    y[b,i] = sum_j cam[i,j]*P1[b,j] + P2[b,i] + bias
    ELU(y) = min(relu(y), exp(y)-1)

v2 design (per core, 43008 rows):
  - x arrives fp16 (host cast; numerically identical to the old SWDGE
    cast-load) -> 22MB HBM reads; y stored fp16, host upcast -> 22MB writes.
  - x must become feature-major for the main matmul. Transpose is split:
    first 31 chunks/SG via xbar DMA transpose, last 32 via PE is_transpose
    matmuls into fp16 PSUM + DVE 2x copies, balancing DMA vs PE time.
  - Epilogue (gpsimd cannot read PSUM): P1 cast-copy split 3:1 Act/DVE,
    exp on Act, (exp-1) on Pool (SBUF fp16), combine = (y max 0) min (exp-1)
    as ONE DVE pass reading PSUM, fp16 out.
  - Software pipeline skew 2 quads (PSUM bufs=3) so the Act P1-copy ->
    cam-matmul chain never stalls the PE (keeps p-state at full clock).
"""

import collections

import numpy as np

import concourse.bass as bass
import concourse.bacc as bacc
import concourse.mybir as mybir
import concourse.tile as tile
from concourse.bass_utils import run_bass_kernel_spmd

N_CORES = 8
N = 21
F = 256
OUT = 256
ROWS_PER_CORE = 43008          # 2048 batches * 21 joints
GROUP_BATCHES = 6
GROUP_ROWS = GROUP_BATCHES * N      # 126
SG_ROWS = 8064                 # lcm(128, 126): 63 chunks, 64 groups
SG_CHUNKS = 63
TAIL_SG_ROWS = 2688            # 43008 - 5*8064: 21 chunks, 21 groups + 42 rows
TAIL_CHUNKS = 21
XBAR_CHUNKS = 31               # per full SG: chunks 0..30 via xbar transpose
PE_CHUNKS = 32                 # chunks 31..62 via PE transpose (4 fills of 8/half)
TAIL_XBAR_CHUNKS = 13
TAIL_PE_CHUNKS = 8
FILL = 8                       # PE transposes per psum fill / DVE copy
STORE_GROUPS = 8               # groups per output store DMA (1008 rows)
QUAD = 2                       # groups per PSUM tile; batches epilogue ops
P1_ON_ACT = (True, True, True, False)  # P1-copy engine split, by quad index

f16 = mybir.dt.float16
f32 = mybir.dt.float32

_ALU = mybir.AluOpType
_ACT = mybir.ActivationFunctionType


def _emit_loads(nc, tiles, x_dram, r0, nxb, npe):
    """Issue the 3 HWDGE loads for one super-group: two feature-half tiles
    for the xbar chunks (256B descs), one row-major tile for the PE chunks
    (512B descs)."""
    xb0, xb1, xrm = tiles
    nc.sync.dma_start(
        xb0[:, 0:nxb, :],
        x_dram[r0 : r0 + nxb * 128, 0:128].rearrange("(c p) f -> p c f", p=128),
    )
    nc.sync.dma_start(
        xb1[:, 0:nxb, :],
        x_dram[r0 : r0 + nxb * 128, 128:256].rearrange("(c p) f -> p c f", p=128),
    )
    rp = r0 + nxb * 128
    nc.sync.dma_start(
        xrm[:, 0:npe, :],
        x_dram[rp : rp + npe * 128, :].rearrange("(c p) f -> p c f", p=128),
    )


def _make_transpose_thunks(nc, pools, consts, ld_tiles, xt_tiles, nxb, npe):
    """Build the list of transpose thunks for one super-group: 2 xbar
    instrs + one (8 PE transposes + DVE copy) thunk per fill."""
    tpppool = pools["tpp"]
    ident_sb = consts["ident"]
    xb0, xb1, xrm = ld_tiles
    xt0, xt1 = xt_tiles
    thunks = []

    def xbar(xb, xt):
        def run():
            nc.sync.dma_start_transpose(
                xt[:, 0 : nxb * 128].rearrange("p (c q) -> p c q", q=128),
                xb[:, 0:nxb, :],
            )
        return run

    thunks.append(xbar(xb0, xt0))
    thunks.append(xbar(xb1, xt1))

    n_fills = npe // FILL
    for fi in range(n_fills):
        for h, xt in ((0, xt0), (1, xt1)):
            def fill(fi=fi, h=h, xt=xt):
                tpp = tpppool.tile([128, FILL, 128], f16, tag="tpp", name="tpp")
                for i in range(FILL):
                    nc.tensor.transpose(
                        tpp[:, i, :],
                        xrm[:, fi * FILL + i, h * 128 : (h + 1) * 128],
                        ident_sb[:, :],
                    )
                c0 = (nxb + fi * FILL) * 128
                nc.vector.tensor_copy(
                    xt[:, c0 : c0 + FILL * 128].rearrange("p (c q) -> p c q", q=128),
                    tpp[:, 0:FILL, :],
                )
            thunks.append(fill)
    return thunks


def _emit_front(nc, pools, consts, xt_tiles, st):
    """Front half of one quad: main matmuls + P1 cast-copy."""
    psumpool = pools["psum"]
    wt0_sb, wt1_sb = consts["wt0"], consts["wt1"]
    p1rot, p1tail = consts["p1rot"], consts["p1tail"]
    xt0, xt1 = xt_tiles
    q0, qn, mrows, is_tail, qidx = (
        st["q0"], st["qn"], st["mrows"], st["is_tail"], st["qidx"])

    p = psumpool.tile([128, QUAD, 512], f32, tag="psum", name="psum")
    st["p"] = p
    for qq in range(qn):
        gr0 = (q0 + qq) * GROUP_ROWS
        nc.tensor.matmul(
            p[0:mrows, qq, 0:512], xt0[:, gr0 : gr0 + mrows], wt0_sb[:, :],
            start=True, stop=False,
        )
        nc.tensor.matmul(
            p[0:mrows, qq, 0:512], xt1[:, gr0 : gr0 + mrows], wt1_sb[:, :],
            start=False, stop=True,
        )

    p1t = p1tail if is_tail else p1rot[qidx % len(p1rot)]
    st["p1t"] = p1t
    if P1_ON_ACT[qidx % len(P1_ON_ACT)] and not is_tail:
        nc.scalar.copy(p1t[0:mrows, 0:qn, :], p[0:mrows, 0:qn, 0:OUT])
    else:
        nc.vector.tensor_copy(p1t[0:mrows, 0:qn, :], p[0:mrows, 0:qn, 0:OUT])


def _emit_back(nc, pools, consts, st, flush):
    """Back half of one quad: cam matmul + ELU + (maybe) store flush."""
    epool, e1pool, ypool = pools["e"], pools["e1"], pools["y"]
    p, p1t = st["p"], st["p1t"]
    q0, qn, mrows, is_tail = st["q0"], st["qn"], st["mrows"], st["is_tail"]
    cam_sb = consts["cam2"] if is_tail else consts["cam6"]
    y_dram = st["y_dram"]

    nc.tensor.matmul(
        p[0:mrows, 0:qn, 256:512],
        cam_sb[0 : mrows + 1, 0:mrows],
        p1t[0 : mrows + 1, 0:qn, :],
        start=False, stop=True, skip_group_check=True,
    )

    # ELU(y) = min(relu(y), exp(y)-1): exp on Act, -1 on Pool, single
    # PSUM-reading combine on DVE.
    esb = epool.tile([128, QUAD, OUT], f16, tag="esb", name="esb")
    nc.scalar.activation(
        esb[0:mrows, 0:qn, :], p[0:mrows, 0:qn, 256:512], _ACT.Exp
    )
    e1t = e1pool.tile([128, QUAD, OUT], f16, tag="e1", name="e1")
    nc.gpsimd.tensor_scalar_sub(e1t[0:mrows, 0:qn, :], esb[0:mrows, 0:qn, :], 1.0)

    if flush["ysb"] is None:
        flush["ysb"] = ypool.tile([128, STORE_GROUPS, OUT], f16, tag="ysb", name="ysb")
        flush["g0"] = q0
        flush["r0"] = st["r0"]
    ysb = flush["ysb"]
    slot = q0 - flush["g0"]
    nc.vector.scalar_tensor_tensor(
        ysb[0:mrows, slot : slot + qn, :],
        p[0:mrows, 0:qn, 256:512], 0.0, e1t[0:mrows, 0:qn, :],
        _ALU.max, _ALU.min,
    )
    if is_tail:
        if slot > 0:
            rf0 = flush["r0"] + flush["g0"] * GROUP_ROWS
            nc.sync.dma_start(
                y_dram[rf0 : rf0 + slot * GROUP_ROWS, :].rearrange(
                    "(g p) f -> p g f", p=GROUP_ROWS
                ),
                ysb[0:GROUP_ROWS, 0:slot, :],
            )
        rt0 = flush["r0"] + q0 * GROUP_ROWS
        nc.sync.dma_start(y_dram[rt0 : rt0 + 42, :], ysb[0:42, slot, :])
        flush["ysb"] = None
    elif slot + qn == STORE_GROUPS:
        rf0 = flush["r0"] + flush["g0"] * GROUP_ROWS
        nc.sync.dma_start(
            y_dram[rf0 : rf0 + (slot + qn) * GROUP_ROWS, :].rearrange(
                "(g p) f -> p g f", p=GROUP_ROWS
            ),
            ysb[0:GROUP_ROWS, 0 : slot + qn, :],
        )
        flush["ysb"] = None


def _build_nc():
    nc = bacc.Bacc("TRN2", target_bir_lowering=False, debug=False,
                   num_devices=N_CORES)
    x_dram = nc.dram_tensor("xs", [ROWS_PER_CORE, F], f16, kind="ExternalInput")
    wt_dram = nc.dram_tensor("wt", [F, 2 * OUT], f16, kind="ExternalInput")
    cam6_dram = nc.dram_tensor("cam6", [128, GROUP_ROWS], f16, kind="ExternalInput")
    cam2_dram = nc.dram_tensor("cam2", [128, 42], f16, kind="ExternalInput")
    bias_dram = nc.dram_tensor("biasr", [QUAD, OUT], f16, kind="ExternalInput")
    ident_dram = nc.dram_tensor("ident", [128, 128], f16, kind="ExternalInput")
    y_dram = nc.dram_tensor("y", [ROWS_PER_CORE, OUT], f16, kind="ExternalOutput")

    with tile.TileContext(nc) as tc:
        with (
            tc.tile_pool(name="consts", bufs=1) as cpool,
            tc.tile_pool(name="xb", bufs=2) as xbpool,
            tc.tile_pool(name="xrm", bufs=2) as xrmpool,
            tc.tile_pool(name="xt", bufs=2) as xtpool,
            tc.tile_pool(name="tpp", bufs=2, space=bass.MemorySpace.PSUM) as tpppool,
            tc.tile_pool(name="psum", bufs=3, space=bass.MemorySpace.PSUM) as psumpool,
            tc.tile_pool(name="e", bufs=3) as epool,
            tc.tile_pool(name="e1", bufs=3) as e1pool,
            tc.tile_pool(name="y", bufs=2) as ypool,
        ):
            wt0_sb = cpool.tile([128, 2 * OUT], f16, tag="wt0")
            wt1_sb = cpool.tile([128, 2 * OUT], f16, tag="wt1")
            cam6_sb = cpool.tile([128, GROUP_ROWS], f16, tag="cam6")
            cam2_sb = cpool.tile([128, 42], f16, tag="cam2")
            ident_sb = cpool.tile([128, 128], f16, tag="ident")
            nc.sync.dma_start(wt0_sb[:, :], wt_dram[0:128, :])
            nc.sync.dma_start(wt1_sb[:, :], wt_dram[128:256, :])
            nc.sync.dma_start(cam6_sb[:, :], cam6_dram[:, :])
            nc.sync.dma_start(cam2_sb[:, :], cam2_dram[:, :])
            nc.sync.dma_start(ident_sb[:, :], ident_dram[:, :])
            # Rotating cam-matmul rhs tiles; bias row (partition GROUP_ROWS /
            # 42 for the tail tile) is written once here and never again.
            p1rot = [cpool.tile([128, QUAD, OUT], f16, tag=f"p1rot{i}",
                                name=f"p1rot{i}")
                     for i in range(4)]
            p1tail = cpool.tile([128, QUAD, OUT], f16, tag="p1tail")
            for t in p1rot:
                nc.sync.dma_start(t[GROUP_ROWS : GROUP_ROWS + 1, :, :],
                                  bias_dram[:, :])
            nc.sync.dma_start(p1tail[42:43, 0:1, :], bias_dram[0:1, :])

            consts = dict(wt0=wt0_sb, wt1=wt1_sb, cam6=cam6_sb, cam2=cam2_sb,
                          ident=ident_sb, p1rot=p1rot, p1tail=p1tail)
            pools = dict(tpp=tpppool, psum=psumpool, e=epool, e1=e1pool,
                         y=ypool)

            # Super-group descriptors: (r0, nxb, npe, n_full_groups, has_tail)
            n_full_sg = ROWS_PER_CORE // SG_ROWS  # 5
            sgs = [(sg * SG_ROWS, XBAR_CHUNKS, PE_CHUNKS, SG_ROWS // GROUP_ROWS,
                    False) for sg in range(n_full_sg)]
            sgs.append((n_full_sg * SG_ROWS, TAIL_XBAR_CHUNKS, TAIL_PE_CHUNKS,
                        (TAIL_SG_ROWS - 42) // GROUP_ROWS, True))

            def ld_tiles(nxb, npe):
                return (
                    xbpool.tile([128, XBAR_CHUNKS, 128], f16, tag="xb0", name="xb0"),
                    xbpool.tile([128, XBAR_CHUNKS, 128], f16, tag="xb1", name="xb1"),
                    xrmpool.tile([128, PE_CHUNKS, 256], f16, tag="xrm", name="xrm"),
                )

            def xt_tiles():
                return (xtpool.tile([128, SG_ROWS], f16, tag="xt0", name="xt0"),
                        xtpool.tile([128, SG_ROWS], f16, tag="xt1", name="xt1"))

            # Prologue: loads + all transposes for SG0.
            lt = ld_tiles(sgs[0][1], sgs[0][2])
            _emit_loads(nc, lt, x_dram, sgs[0][0], sgs[0][1], sgs[0][2])
            xt = xt_tiles()
            for th in _make_transpose_thunks(nc, pools, consts, lt, xt,
                                             sgs[0][1], sgs[0][2]):
                th()

            flush = {"ysb": None}
            pending = collections.deque()
            qidx = 0
            for si, (r0, nxb, npe, n_full, has_tail) in enumerate(sgs):
                # Issue next SG's loads and build its transpose thunks.
                nxt_thunks = []
                if si + 1 < len(sgs):
                    nr0, nnxb, nnpe, _, _ = sgs[si + 1]
                    nlt = ld_tiles(nnxb, nnpe)
                    _emit_loads(nc, nlt, x_dram, nr0, nnxb, nnpe)
                    nxt = xt_tiles()
                    nxt_thunks = _make_transpose_thunks(
                        nc, pools, consts, nlt, nxt, nnxb, nnpe)

                quads = [(t0, min(QUAD, n_full - t0))
                         for t0 in range(0, n_full, QUAD)]
                if has_tail:
                    quads.append((n_full, -1))

                # Drain next-SG transpose thunks spread over the middle of
                # this SG's quad loop (loads have landed by then; xbar first
                # so the next SG's first groups are ready earliest).
                n_q = len(quads)
                sched = []
                if nxt_thunks:
                    d0 = min(8, max(0, n_q - len(nxt_thunks)))
                    span = max(1, n_q - d0)
                    sched = [d0 + (k * span) // len(nxt_thunks)
                             for k in range(len(nxt_thunks))]
                for qi, (t0, tn) in enumerate(quads):
                    while sched and sched[0] <= qi:
                        sched.pop(0)
                        nxt_thunks.pop(0)()
                    is_tail = tn == -1
                    st = dict(
                        q0=t0, qn=1 if is_tail else tn,
                        mrows=42 if is_tail else GROUP_ROWS,
                        is_tail=is_tail, qidx=qidx, r0=r0, y_dram=y_dram,
                    )
                    _emit_front(nc, pools, consts, xt, st)
                    if len(pending) == 2:
                        _emit_back(nc, pools, consts, pending.popleft(), flush)
                    pending.append(st)
                    qidx += 1
                for th in nxt_thunks:
                    th()
                if si + 1 < len(sgs):
                    xt = nxt

            while pending:
                _emit_back(nc, pools, consts, pending.popleft(), flush)

    nc.compile()
    return nc


_NC_CACHE = None


def _host_constants(cam, W, b):
    W = np.asarray(W, np.float32)
    cam = np.asarray(cam, np.float32)
    b = np.asarray(b, np.float32)
    # rhs of matmul1: [f, o2] with o2<256 -> W1.T, o2>=256 -> W2.T
    wt = np.concatenate([W[:, :F].T, W[:, F:].T], axis=1).astype(np.float16)
    # Block-diagonal cam.T (6 batches) + ones row for the bias term.
    cam6 = np.zeros((128, GROUP_ROWS), np.float32)
    for bb in range(GROUP_BATCHES):
        cam6[bb * N : (bb + 1) * N, bb * N : (bb + 1) * N] = cam.T
    cam6[GROUP_ROWS, :] = 1.0
    cam2 = np.zeros((128, 42), np.float32)
    for bb in range(2):
        cam2[bb * N : (bb + 1) * N, bb * N : (bb + 1) * N] = cam.T
    cam2[42, :] = 1.0
    biasr = np.tile(b.reshape(1, OUT), (QUAD, 1))
    ident = np.eye(128, dtype=np.float16)
    return (wt, cam6.astype(np.float16), cam2.astype(np.float16),
            biasr.astype(np.float16), ident)


def make_in_maps(x, cam, W, b):
    x16 = np.ascontiguousarray(np.asarray(x)).astype(np.float16)
    assert x16.shape == (N_CORES * ROWS_PER_CORE, F)
    wt, cam6, cam2, biasr, ident = _host_constants(cam, W, b)
    in_maps = []
    for i in range(N_CORES):
        in_maps.append({
            "xs": x16[i * ROWS_PER_CORE : (i + 1) * ROWS_PER_CORE, :],
            "wt": wt, "cam6": cam6, "cam2": cam2, "biasr": biasr,
            "ident": ident,
        })
    return in_maps


def kernel(x, cam, W, b, n_joints):
    global _NC_CACHE
    if _NC_CACHE is None:
        _NC_CACHE = _build_nc()
    nc = _NC_CACHE
    in_maps = make_in_maps(x, cam, W, b)
    res = run_bass_kernel_spmd(nc, in_maps, core_ids=list(range(N_CORES)))
    y = np.concatenate([res.results[i]["y"] for i in range(N_CORES)], axis=0)
    return y.astype(np.float32)


# revision 9
# speedup vs baseline: 3.8488x; 3.8488x over previous
"""CAM graph layer (message passing + Linear + ELU) on 8 Trainium2 NeuronCores.

Reference computation (per batch b of N=21 joints, F=256 features):
    x_agg[b,i] = sum_j cam[i,j] * x[b,j]            (21x21 aggregation)
    y = ELU(concat([x_agg, x], -1) @ W.T + b)       (Linear 512->256)

Kernel restructure:
    P1 = x @ W1.T ; P2 = x @ W2.T          (W1/W2 = halves of W)
    y[b,i] = sum_j cam[i,j]*P1[b,j] + P2[b,i] + bias
    ELU(y) = min(relu(y), exp(y)-1)

v3 design (per core, 43008 rows):
  - x arrives fp16 (host cast; numerically identical to the old SWDGE
    cast-load) -> 22MB HBM reads; y stored fp16, host upcast -> 22MB writes.
  - x must become feature-major for the matmuls. Transpose split: first 31
    chunks/SG via xbar DMA transpose, last 32 via PE is_transpose matmuls
    into fp16 PSUM + DVE 2x copies, balancing DMA vs PE time.
  - QUAD=4 groups per PSUM tile to amortize the fixed SBUF/PSUM access
    latency of every epilogue instruction. PSUM (8 banks): P1 pool
    [128,4,256]f32 bufs=1 (2) + Y pool [128,4,256]f32 bufs=2 (4) + PE
    transpose staging [128,8,128]f16 bufs=2 (2).
  - Epilogue engines (gpsimd cannot read PSUM and its ALU is ~15x slower
    than DVE -- never give it bulk work): P1 cast-copy + exp on Act,
    (exp-1) 4x + combine (y max 0) min (exp-1) + transpose copies on DVE.
  - Software pipeline skew 1 quad: PE order per quad is
    [P1-matmuls(q), P2-matmuls(q), cam-matmul(q-1)], so the Act P1-copy(q)
    overlaps the P2 matmuls and the cam matmul of the previous quad.
"""

import collections

import numpy as np

import concourse.bass as bass
import concourse.bacc as bacc
import concourse.mybir as mybir
import concourse.tile as tile
from concourse.bass_utils import run_bass_kernel_spmd

N_CORES = 8
N = 21
F = 256
OUT = 256
ROWS_PER_CORE = 43008          # 2048 batches * 21 joints
GROUP_BATCHES = 6
GROUP_ROWS = GROUP_BATCHES * N      # 126
SG_ROWS = 8064                 # lcm(128, 126): 63 chunks, 64 groups
TAIL_SG_ROWS = 2688            # 43008 - 5*8064: 21 chunks, 21 groups + 42 rows
XBAR_CHUNKS = 31               # per full SG: chunks 0..30 via xbar transpose
PE_CHUNKS = 32                 # chunks 31..62 via PE transpose (4 fills of 8/half)
TAIL_XBAR_CHUNKS = 13
TAIL_PE_CHUNKS = 8
FILL = 8                       # PE transposes per psum fill / DVE copy
STORE_GROUPS = 8               # groups per output store DMA (1008 rows)
QUAD = 4                       # groups per PSUM tile; batches epilogue ops

f16 = mybir.dt.float16
f32 = mybir.dt.float32

_ALU = mybir.AluOpType
_ACT = mybir.ActivationFunctionType


def _emit_loads(nc, tiles, x_dram, r0, nxb, npe):
    """Issue the SWDGE loads for one super-group on the idle gpsimd queue
    (keeps the SP queue free for the xbar transposes, whose semaphore wait
    would otherwise head-of-line-block every later SP DMA). PE-transposed
    row-major chunks come FIRST in the SG (512B descs, 2 instrs so the
    first fills can start early); xbar feature-half tiles follow (256B)."""
    xb0, xb1, xrm = tiles
    half = (npe // 2) * 128
    nc.gpsimd.dma_start(
        xrm[:, 0 : npe // 2, :],
        x_dram[r0 : r0 + half, :].rearrange("(c p) f -> p c f", p=128),
    )
    nc.gpsimd.dma_start(
        xrm[:, npe // 2 : npe, :],
        x_dram[r0 + half : r0 + npe * 128, :].rearrange("(c p) f -> p c f", p=128),
    )
    rx = r0 + npe * 128
    nc.gpsimd.dma_start(
        xb0[:, 0:nxb, :],
        x_dram[rx : rx + nxb * 128, 0:128].rearrange("(c p) f -> p c f", p=128),
    )
    nc.gpsimd.dma_start(
        xb1[:, 0:nxb, :],
        x_dram[rx : rx + nxb * 128, 128:256].rearrange("(c p) f -> p c f", p=128),
    )


def _make_transpose_thunks(nc, pools, consts, ld_tiles, xt_tiles, nxb, npe):
    """Build the list of transpose thunks for one super-group: 2 xbar
    instrs + one (8 PE transposes + DVE copy) thunk per fill."""
    tpppool = pools["tpp"]
    ident_sb = consts["ident"]
    xb0, xb1, xrm = ld_tiles
    xt0, xt1 = xt_tiles
    thunks = []

    def xbar(xb, xt):
        def run():
            nc.sync.dma_start_transpose(
                xt[:, 0 : nxb * 128].rearrange("p (c q) -> p c q", q=128),
                xb[:, 0:nxb, :],
            )
        return run

    thunks.append(xbar(xb0, xt0))
    thunks.append(xbar(xb1, xt1))

    n_fills = npe // FILL
    for fi in range(n_fills):
        for h, xt in ((0, xt0), (1, xt1)):
            def fill(fi=fi, h=h, xt=xt):
                tpp = tpppool.tile([128, FILL, 128], f16, tag="tpp", name="tpp")
                for i in range(FILL):
                    nc.tensor.transpose(
                        tpp[:, i, :],
                        xrm[:, fi * FILL + i, h * 128 : (h + 1) * 128],
                        ident_sb[:, :],
                    )
                c0 = (nxb + fi * FILL) * 128
                nc.vector.tensor_copy(
                    xt[:, c0 : c0 + FILL * 128].rearrange("p (c q) -> p c q", q=128),
                    tpp[:, 0:FILL, :],
                )
            thunks.append(fill)
    return thunks


def _emit_front(nc, pools, consts, xt_tiles, st):
    """Front half of one quad: P1 matmuls + Act P1 cast-copy + P2 matmuls."""
    wt0_sb, wt1_sb = consts["wt0"], consts["wt1"]
    p1rot, p1tail = consts["p1rot"], consts["p1tail"]
    xt0, xt1 = xt_tiles
    q0, qn, mrows, is_tail, qidx = (
        st["q0"], st["qn"], st["mrows"], st["is_tail"], st["qidx"])

    pp1 = pools["pp1"].tile([128, QUAD, OUT], f32, tag="pp1", name="pp1")
    py = pools["py"].tile([128, QUAD, OUT], f32, tag="py", name="py")
    st["py"] = py
    # start=True resets the WHOLE 2KB PSUM bank (2 group slots), so only the
    # first matmul touching a bank may set it; stop on the bank's last.
    for qq in range(qn):
        gr0 = (q0 + qq) * GROUP_ROWS
        first = qq % 2 == 0
        last = qq % 2 == 1 or qq == qn - 1
        nc.tensor.matmul(
            pp1[0:mrows, qq, :], xt0[:, gr0 : gr0 + mrows], wt0_sb[:, 0:OUT],
            start=first, stop=False, skip_group_check=True,
        )
        nc.tensor.matmul(
            pp1[0:mrows, qq, :], xt1[:, gr0 : gr0 + mrows], wt1_sb[:, 0:OUT],
            start=False, stop=last, skip_group_check=True,
        )

    p1t = p1tail if is_tail else p1rot[qidx % len(p1rot)]
    st["p1t"] = p1t
    nc.scalar.copy(p1t[0:mrows, 0:qn, :], pp1[0:mrows, 0:qn, :])

    for qq in range(qn):
        gr0 = (q0 + qq) * GROUP_ROWS
        first = qq % 2 == 0
        last = qq % 2 == 1 or qq == qn - 1
        nc.tensor.matmul(
            py[0:mrows, qq, :], xt0[:, gr0 : gr0 + mrows], wt0_sb[:, OUT : 2 * OUT],
            start=first, stop=False, skip_group_check=True,
        )
        nc.tensor.matmul(
            py[0:mrows, qq, :], xt1[:, gr0 : gr0 + mrows], wt1_sb[:, OUT : 2 * OUT],
            start=False, stop=last, skip_group_check=True,
        )


def _emit_back(nc, pools, consts, st, flush):
    """Back half of one quad: cam matmul + ELU + (maybe) store flush."""
    epool, e1pool, ypool = pools["e"], pools["e1"], pools["y"]
    py, p1t = st["py"], st["p1t"]
    q0, qn, mrows, is_tail = st["q0"], st["qn"], st["mrows"], st["is_tail"]
    cam_sb = consts["cam2"] if is_tail else consts["cam6"]
    y_dram = st["y_dram"]

    # Matmul out must stay within one PSUM bank (512 fp32): <=2 groups/instr.
    for h0 in range(0, qn, 2):
        hn = min(2, qn - h0)
        nc.tensor.matmul(
            py[0:mrows, h0 : h0 + hn, :],
            cam_sb[0 : mrows + 1, 0:mrows],
            p1t[0 : mrows + 1, h0 : h0 + hn, :],
            start=False, stop=True, skip_group_check=True,
        )

    # ELU(y) = min(relu(y), exp(y)-1): exp on Act, -1 on DVE 4x, single
    # PSUM-reading combine on DVE.
    esb = epool.tile([128, QUAD, OUT], f16, tag="esb", name="esb")
    nc.scalar.activation(
        esb[0:mrows, 0:qn, :], py[0:mrows, 0:qn, :], _ACT.Exp
    )
    e1t = e1pool.tile([128, QUAD, OUT], f16, tag="e1", name="e1")
    nc.vector.tensor_scalar_sub(e1t[0:mrows, 0:qn, :], esb[0:mrows, 0:qn, :], 1.0)

    if flush["ysb"] is None:
        flush["ysb"] = ypool.tile([128, STORE_GROUPS, OUT], f16, tag="ysb",
                                  name="ysb")
        flush["g0"] = q0
        flush["r0"] = st["r0"]
    ysb = flush["ysb"]
    slot = q0 - flush["g0"]
    nc.vector.scalar_tensor_tensor(
        ysb[0:mrows, slot : slot + qn, :],
        py[0:mrows, 0:qn, :], 0.0, e1t[0:mrows, 0:qn, :],
        _ALU.max, _ALU.min,
    )
    if is_tail:
        if slot > 0:
            rf0 = flush["r0"] + flush["g0"] * GROUP_ROWS
            nc.sync.dma_start(
                y_dram[rf0 : rf0 + slot * GROUP_ROWS, :].rearrange(
                    "(g p) f -> p g f", p=GROUP_ROWS
                ),
                ysb[0:GROUP_ROWS, 0:slot, :],
            )
        rt0 = flush["r0"] + q0 * GROUP_ROWS
        nc.sync.dma_start(y_dram[rt0 : rt0 + 42, :], ysb[0:42, slot, :])
        flush["ysb"] = None
    elif slot + qn == STORE_GROUPS:
        rf0 = flush["r0"] + flush["g0"] * GROUP_ROWS
        nc.sync.dma_start(
            y_dram[rf0 : rf0 + (slot + qn) * GROUP_ROWS, :].rearrange(
                "(g p) f -> p g f", p=GROUP_ROWS
            ),
            ysb[0:GROUP_ROWS, 0 : slot + qn, :],
        )
        flush["ysb"] = None


def _build_nc():
    nc = bacc.Bacc("TRN2", target_bir_lowering=False, debug=False,
                   num_devices=N_CORES)
    x_dram = nc.dram_tensor("xs", [ROWS_PER_CORE, F], f16, kind="ExternalInput")
    wt_dram = nc.dram_tensor("wt", [F, 2 * OUT], f16, kind="ExternalInput")
    cam6_dram = nc.dram_tensor("cam6", [128, GROUP_ROWS], f16, kind="ExternalInput")
    cam2_dram = nc.dram_tensor("cam2", [128, 42], f16, kind="ExternalInput")
    bias_dram = nc.dram_tensor("biasr", [QUAD, OUT], f16, kind="ExternalInput")
    ident_dram = nc.dram_tensor("ident", [128, 128], f16, kind="ExternalInput")
    y_dram = nc.dram_tensor("y", [ROWS_PER_CORE, OUT], f16, kind="ExternalOutput")

    with tile.TileContext(nc) as tc:
        with (
            tc.tile_pool(name="consts", bufs=1) as cpool,
            tc.tile_pool(name="xb", bufs=2) as xbpool,
            tc.tile_pool(name="xrm", bufs=2) as xrmpool,
            tc.tile_pool(name="xt", bufs=2) as xtpool,
            tc.tile_pool(name="tpp", bufs=2, space=bass.MemorySpace.PSUM) as tpppool,
            tc.tile_pool(name="pp1", bufs=1, space=bass.MemorySpace.PSUM) as pp1pool,
            tc.tile_pool(name="py", bufs=2, space=bass.MemorySpace.PSUM) as pypool,
            tc.tile_pool(name="e", bufs=3) as epool,
            tc.tile_pool(name="e1", bufs=3) as e1pool,
            tc.tile_pool(name="y", bufs=2) as ypool,
        ):
            wt0_sb = cpool.tile([128, 2 * OUT], f16, tag="wt0")
            wt1_sb = cpool.tile([128, 2 * OUT], f16, tag="wt1")
            cam6_sb = cpool.tile([128, GROUP_ROWS], f16, tag="cam6")
            cam2_sb = cpool.tile([128, 42], f16, tag="cam2")
            ident_sb = cpool.tile([128, 128], f16, tag="ident")
            nc.sync.dma_start(wt0_sb[:, :], wt_dram[0:128, :])
            nc.sync.dma_start(wt1_sb[:, :], wt_dram[128:256, :])
            nc.sync.dma_start(cam6_sb[:, :], cam6_dram[:, :])
            nc.sync.dma_start(cam2_sb[:, :], cam2_dram[:, :])
            nc.sync.dma_start(ident_sb[:, :], ident_dram[:, :])
            # Rotating cam-matmul rhs tiles; bias row (partition GROUP_ROWS /
            # 42 for the tail tile) is written once here and never again.
            p1rot = [cpool.tile([128, QUAD, OUT], f16, tag=f"p1rot{i}",
                                name=f"p1rot{i}")
                     for i in range(4)]
            p1tail = cpool.tile([128, QUAD, OUT], f16, tag="p1tail")
            for t in p1rot:
                nc.sync.dma_start(t[GROUP_ROWS : GROUP_ROWS + 1, :, :],
                                  bias_dram[:, :])
            nc.sync.dma_start(p1tail[42:43, 0:1, :], bias_dram[0:1, :])

            consts = dict(wt0=wt0_sb, wt1=wt1_sb, cam6=cam6_sb, cam2=cam2_sb,
                          ident=ident_sb, p1rot=p1rot, p1tail=p1tail)
            pools = dict(tpp=tpppool, pp1=pp1pool, py=pypool, e=epool,
                         e1=e1pool, y=ypool)

            # Super-group descriptors: (r0, nxb, npe, n_full_groups, has_tail)
            n_full_sg = ROWS_PER_CORE // SG_ROWS  # 5
            sgs = [(sg * SG_ROWS, XBAR_CHUNKS, PE_CHUNKS, SG_ROWS // GROUP_ROWS,
                    False) for sg in range(n_full_sg)]
            sgs.append((n_full_sg * SG_ROWS, TAIL_XBAR_CHUNKS, TAIL_PE_CHUNKS,
                        (TAIL_SG_ROWS - 42) // GROUP_ROWS, True))

            def ld_tiles():
                return (
                    xbpool.tile([128, XBAR_CHUNKS, 128], f16, tag="xb0",
                                name="xb0"),
                    xbpool.tile([128, XBAR_CHUNKS, 128], f16, tag="xb1",
                                name="xb1"),
                    xrmpool.tile([128, PE_CHUNKS, 256], f16, tag="xrm",
                                 name="xrm"),
                )

            def xt_tiles():
                return (xtpool.tile([128, SG_ROWS], f16, tag="xt0", name="xt0"),
                        xtpool.tile([128, SG_ROWS], f16, tag="xt1", name="xt1"))

            # Prologue: loads + all transposes for SG0.
            lt = ld_tiles()
            _emit_loads(nc, lt, x_dram, sgs[0][0], sgs[0][1], sgs[0][2])
            xt = xt_tiles()
            for th in _make_transpose_thunks(nc, pools, consts, lt, xt,
                                             sgs[0][1], sgs[0][2]):
                th()

            flush = {"ysb": None}
            pending = collections.deque()
            qidx = 0
            for si, (r0, nxb, npe, n_full, has_tail) in enumerate(sgs):
                # Issue next SG's loads and build its transpose thunks.
                nxt_thunks = []
                if si + 1 < len(sgs):
                    nr0, nnxb, nnpe, _, _ = sgs[si + 1]
                    nlt = ld_tiles()
                    _emit_loads(nc, nlt, x_dram, nr0, nnxb, nnpe)
                    nxt = xt_tiles()
                    nxt_thunks = _make_transpose_thunks(
                        nc, pools, consts, nlt, nxt, nnxb, nnpe)

                quads = [(t0, min(QUAD, n_full - t0))
                         for t0 in range(0, n_full, QUAD)]
                if has_tail:
                    quads.append((n_full, -1))

                # Drain next-SG transpose thunks spread over the middle of
                # this SG's quad loop (loads have landed by then; xbar first
                # so the next SG's first groups are ready earliest).
                n_q = len(quads)
                sched = []
                if nxt_thunks:
                    d0 = min(4, max(0, n_q - len(nxt_thunks)))
                    span = max(1, n_q - d0)
                    sched = [d0 + (k * span) // len(nxt_thunks)
                             for k in range(len(nxt_thunks))]
                for qi, (t0, tn) in enumerate(quads):
                    while sched and sched[0] <= qi:
                        sched.pop(0)
                        nxt_thunks.pop(0)()
                    is_tail = tn == -1
                    st = dict(
                        q0=t0, qn=1 if is_tail else tn,
                        mrows=42 if is_tail else GROUP_ROWS,
                        is_tail=is_tail, qidx=qidx, r0=r0, y_dram=y_dram,
                    )
                    _emit_front(nc, pools, consts, xt, st)
                    if pending:
                        _emit_back(nc, pools, consts, pending.popleft(), flush)
                    pending.append(st)
                    qidx += 1
                for th in nxt_thunks:
                    th()
                if si + 1 < len(sgs):
                    xt = nxt

            while pending:
                _emit_back(nc, pools, consts, pending.popleft(), flush)

    nc.compile()
    return nc


_NC_CACHE = None


def _host_constants(cam, W, b):
    W = np.asarray(W, np.float32)
    cam = np.asarray(cam, np.float32)
    b = np.asarray(b, np.float32)
    # rhs of matmuls: [f, o2] with o2<256 -> W1.T, o2>=256 -> W2.T
    wt = np.concatenate([W[:, :F].T, W[:, F:].T], axis=1).astype(np.float16)
    # Block-diagonal cam.T (6 batches) + ones row for the bias term.
    cam6 = np.zeros((128, GROUP_ROWS), np.float32)
    for bb in range(GROUP_BATCHES):
        cam6[bb * N : (bb + 1) * N, bb * N : (bb + 1) * N] = cam.T
    cam6[GROUP_ROWS, :] = 1.0
    cam2 = np.zeros((128, 42), np.float32)
    for bb in range(2):
        cam2[bb * N : (bb + 1) * N, bb * N : (bb + 1) * N] = cam.T
    cam2[42, :] = 1.0
    biasr = np.tile(b.reshape(1, OUT), (QUAD, 1))
    ident = np.eye(128, dtype=np.float16)
    return (wt, cam6.astype(np.float16), cam2.astype(np.float16),
            biasr.astype(np.float16), ident)


def make_in_maps(x, cam, W, b):
    x16 = np.ascontiguousarray(np.asarray(x)).astype(np.float16)
    assert x16.shape == (N_CORES * ROWS_PER_CORE, F)
    wt, cam6, cam2, biasr, ident = _host_constants(cam, W, b)
    in_maps = []
    for i in range(N_CORES):
        in_maps.append({
            "xs": x16[i * ROWS_PER_CORE : (i + 1) * ROWS_PER_CORE, :],
            "wt": wt, "cam6": cam6, "cam2": cam2, "biasr": biasr,
            "ident": ident,
        })
    return in_maps


def kernel(x, cam, W, b, n_joints):
    global _NC_CACHE
    if _NC_CACHE is None:
        _NC_CACHE = _build_nc()
    nc = _NC_CACHE
    in_maps = make_in_maps(x, cam, W, b)
    res = run_bass_kernel_spmd(nc, in_maps, core_ids=list(range(N_CORES)))
    y = np.concatenate([res.results[i]["y"] for i in range(N_CORES)], axis=0)
    return y.astype(np.float32)


# revision 11
# speedup vs baseline: 3.9099x; 1.0159x over previous
"""CAM graph layer (message passing + Linear + ELU) on 8 Trainium2 NeuronCores.

Reference computation (per batch b of N=21 joints, F=256 features):
    x_agg[b,i] = sum_j cam[i,j] * x[b,j]            (21x21 aggregation)
    y = ELU(concat([x_agg, x], -1) @ W.T + b)       (Linear 512->256)

Kernel restructure:
    P1 = x @ W1.T ; P2 = x @ W2.T          (W1/W2 = halves of W)
    y[b,i] = sum_j cam[i,j]*P1[b,j] + P2[b,i] + bias
    ELU(y) = min(relu(y), exp(y)-1)

v3 design (per core, 43008 rows):
  - x arrives fp16 (host cast; numerically identical to the old SWDGE
    cast-load) -> 22MB HBM reads; y stored fp16, host upcast -> 22MB writes.
  - x must become feature-major for the matmuls. Transpose split: first 31
    chunks/SG via xbar DMA transpose, last 32 via PE is_transpose matmuls
    into fp16 PSUM + DVE 2x copies, balancing DMA vs PE time.
  - QUAD=4 groups per PSUM tile to amortize the fixed SBUF/PSUM access
    latency of every epilogue instruction. PSUM (8 banks): P1 pool
    [128,4,256]f32 bufs=1 (2) + Y pool [128,4,256]f32 bufs=2 (4) + PE
    transpose staging [128,8,128]f16 bufs=2 (2).
  - Epilogue engines (gpsimd cannot read PSUM and its ALU is ~15x slower
    than DVE -- never give it bulk work): P1 cast-copy + exp on Act,
    (exp-1) 4x + combine (y max 0) min (exp-1) + transpose copies on DVE.
  - Software pipeline skew 1 quad: PE order per quad is
    [P1-matmuls(q), P2-matmuls(q), cam-matmul(q-1)], so the Act P1-copy(q)
    overlaps the P2 matmuls and the cam matmul of the previous quad.
"""

import collections

import numpy as np

import concourse.bass as bass
import concourse.bacc as bacc
import concourse.mybir as mybir
import concourse.tile as tile
from concourse.bass_utils import run_bass_kernel_spmd

N_CORES = 8
N = 21
F = 256
OUT = 256
ROWS_PER_CORE = 43008          # 2048 batches * 21 joints
GROUP_BATCHES = 6
GROUP_ROWS = GROUP_BATCHES * N      # 126
SG_ROWS = 8064                 # lcm(128, 126): 63 chunks, 64 groups
TAIL_SG_ROWS = 2688            # 43008 - 5*8064: 21 chunks, 21 groups + 42 rows
XBAR_CHUNKS = 31               # per full SG: chunks 0..30 via xbar transpose
PE_CHUNKS = 32                 # chunks 31..62 via PE transpose (4 fills of 8/half)
TAIL_XBAR_CHUNKS = 13
TAIL_PE_CHUNKS = 8
FILL = 8                       # PE transposes per psum fill / DVE copy
STORE_GROUPS = 8               # groups per output store DMA (1008 rows)
QUAD = 4                       # groups per PSUM tile; batches epilogue ops

f16 = mybir.dt.float16
f32 = mybir.dt.float32

_ALU = mybir.AluOpType
_ACT = mybir.ActivationFunctionType


def _emit_loads(nc, tiles, x_dram, r0, nxb, npe):
    """Issue the SWDGE loads for one super-group on the idle gpsimd queue
    (keeps the SP queue free for the xbar transposes, whose semaphore wait
    would otherwise head-of-line-block every later SP DMA). PE-transposed
    row-major chunks come FIRST in the SG (512B descs, 2 instrs so the
    first fills can start early); xbar feature-half tiles follow (256B)."""
    xb0, xb1, xrm = tiles
    half = (npe // 2) * 128
    nc.gpsimd.dma_start(
        xrm[:, 0 : npe // 2, :],
        x_dram[r0 : r0 + half, :].rearrange("(c p) f -> p c f", p=128),
    )
    nc.gpsimd.dma_start(
        xrm[:, npe // 2 : npe, :],
        x_dram[r0 + half : r0 + npe * 128, :].rearrange("(c p) f -> p c f", p=128),
    )
    rx = r0 + npe * 128
    nc.gpsimd.dma_start(
        xb0[:, 0:nxb, :],
        x_dram[rx : rx + nxb * 128, 0:128].rearrange("(c p) f -> p c f", p=128),
    )
    nc.gpsimd.dma_start(
        xb1[:, 0:nxb, :],
        x_dram[rx : rx + nxb * 128, 128:256].rearrange("(c p) f -> p c f", p=128),
    )


def _make_transpose_thunks(nc, pools, consts, ld_tiles, xt_tiles, nxb, npe):
    """Build the list of transpose thunks for one super-group: 2 xbar
    instrs + one (8 PE transposes + DVE copy) thunk per fill."""
    tpppool = pools["tpp"]
    ident_sb = consts["ident"]
    xb0, xb1, xrm = ld_tiles
    xt0, xt1 = xt_tiles
    thunks = []

    def xbar(xb, xt):
        def run():
            c0 = npe * 128
            nc.sync.dma_start_transpose(
                xt[:, c0 : c0 + nxb * 128].rearrange("p (c q) -> p c q", q=128),
                xb[:, 0:nxb, :],
            )
        return run

    n_fills = npe // FILL
    for fi in range(n_fills):
        for h, xt in ((0, xt0), (1, xt1)):
            def fill(fi=fi, h=h, xt=xt):
                tpp = tpppool.tile([128, FILL, 128], f16, tag="tpp", name="tpp")
                for i in range(FILL):
                    nc.tensor.transpose(
                        tpp[:, i, :],
                        xrm[:, fi * FILL + i, h * 128 : (h + 1) * 128],
                        ident_sb[:, :],
                    )
                c0 = fi * FILL * 128
                nc.vector.tensor_copy(
                    xt[:, c0 : c0 + FILL * 128].rearrange("p (c q) -> p c q", q=128),
                    tpp[:, 0:FILL, :],
                )
            thunks.append(fill)

    thunks.append(xbar(xb0, xt0))
    thunks.append(xbar(xb1, xt1))
    return thunks


def _emit_front(nc, pools, consts, xt_tiles, st):
    """Front half of one quad: P1 matmuls + Act P1 cast-copy + P2 matmuls."""
    wt0_sb, wt1_sb = consts["wt0"], consts["wt1"]
    p1rot, p1tail = consts["p1rot"], consts["p1tail"]
    xt0, xt1 = xt_tiles
    q0, qn, mrows, is_tail, qidx = (
        st["q0"], st["qn"], st["mrows"], st["is_tail"], st["qidx"])

    pp1 = pools["pp1"].tile([128, QUAD, OUT], f32, tag="pp1", name="pp1")
    py = pools["py"].tile([128, QUAD, OUT], f32, tag="py", name="py")
    st["py"] = py
    # start=True resets the WHOLE 2KB PSUM bank (2 group slots), so only the
    # first matmul touching a bank may set it; stop on the bank's last.
    for qq in range(qn):
        gr0 = (q0 + qq) * GROUP_ROWS
        first = qq % 2 == 0
        last = qq % 2 == 1 or qq == qn - 1
        nc.tensor.matmul(
            pp1[0:mrows, qq, :], xt0[:, gr0 : gr0 + mrows], wt0_sb[:, 0:OUT],
            start=first, stop=False, skip_group_check=True,
        )
        nc.tensor.matmul(
            pp1[0:mrows, qq, :], xt1[:, gr0 : gr0 + mrows], wt1_sb[:, 0:OUT],
            start=False, stop=last, skip_group_check=True,
        )

    p1t = p1tail if is_tail else p1rot[qidx % len(p1rot)]
    st["p1t"] = p1t
    nc.scalar.copy(p1t[0:mrows, 0:qn, :], pp1[0:mrows, 0:qn, :])

    for qq in range(qn):
        gr0 = (q0 + qq) * GROUP_ROWS
        first = qq % 2 == 0
        last = qq % 2 == 1 or qq == qn - 1
        nc.tensor.matmul(
            py[0:mrows, qq, :], xt0[:, gr0 : gr0 + mrows], wt0_sb[:, OUT : 2 * OUT],
            start=first, stop=False, skip_group_check=True,
        )
        nc.tensor.matmul(
            py[0:mrows, qq, :], xt1[:, gr0 : gr0 + mrows], wt1_sb[:, OUT : 2 * OUT],
            start=False, stop=last, skip_group_check=True,
        )


def _emit_back(nc, pools, consts, st, flush):
    """Back half of one quad: cam matmul + ELU + (maybe) store flush."""
    epool, e1pool, ypool = pools["e"], pools["e1"], pools["y"]
    py, p1t = st["py"], st["p1t"]
    q0, qn, mrows, is_tail = st["q0"], st["qn"], st["mrows"], st["is_tail"]
    cam_sb = consts["cam2"] if is_tail else consts["cam6"]
    y_dram = st["y_dram"]

    # Matmul out must stay within one PSUM bank (512 fp32): <=2 groups/instr.
    for h0 in range(0, qn, 2):
        hn = min(2, qn - h0)
        nc.tensor.matmul(
            py[0:mrows, h0 : h0 + hn, :],
            cam_sb[0 : mrows + 1, 0:mrows],
            p1t[0 : mrows + 1, h0 : h0 + hn, :],
            start=False, stop=True, skip_group_check=True,
        )

    # ELU(y) = min(relu(y), exp(y)-1): exp on Act, -1 on DVE 4x, single
    # PSUM-reading combine on DVE.
    esb = epool.tile([128, QUAD, OUT], f16, tag="esb", name="esb")
    nc.scalar.activation(
        esb[0:mrows, 0:qn, :], py[0:mrows, 0:qn, :], _ACT.Exp
    )
    e1t = e1pool.tile([128, QUAD, OUT], f16, tag="e1", name="e1")
    nc.vector.tensor_scalar_sub(e1t[0:mrows, 0:qn, :], esb[0:mrows, 0:qn, :], 1.0)

    if flush["ysb"] is None:
        flush["ysb"] = ypool.tile([128, STORE_GROUPS, OUT], f16, tag="ysb",
                                  name="ysb")
        flush["g0"] = q0
        flush["r0"] = st["r0"]
    ysb = flush["ysb"]
    slot = q0 - flush["g0"]
    nc.vector.scalar_tensor_tensor(
        ysb[0:mrows, slot : slot + qn, :],
        py[0:mrows, 0:qn, :], 0.0, e1t[0:mrows, 0:qn, :],
        _ALU.max, _ALU.min,
    )
    if is_tail:
        if slot > 0:
            rf0 = flush["r0"] + flush["g0"] * GROUP_ROWS
            nc.scalar.dma_start(
                y_dram[rf0 : rf0 + slot * GROUP_ROWS, :].rearrange(
                    "(g p) f -> p g f", p=GROUP_ROWS
                ),
                ysb[0:GROUP_ROWS, 0:slot, :],
            )
        rt0 = flush["r0"] + q0 * GROUP_ROWS
        nc.scalar.dma_start(y_dram[rt0 : rt0 + 42, :], ysb[0:42, slot, :])
        flush["ysb"] = None
    elif slot + qn == STORE_GROUPS:
        rf0 = flush["r0"] + flush["g0"] * GROUP_ROWS
        nc.scalar.dma_start(
            y_dram[rf0 : rf0 + (slot + qn) * GROUP_ROWS, :].rearrange(
                "(g p) f -> p g f", p=GROUP_ROWS
            ),
            ysb[0:GROUP_ROWS, 0 : slot + qn, :],
        )
        flush["ysb"] = None


def _build_nc():
    nc = bacc.Bacc("TRN2", target_bir_lowering=False, debug=False,
                   num_devices=N_CORES)
    x_dram = nc.dram_tensor("xs", [ROWS_PER_CORE, F], f16, kind="ExternalInput")
    wt_dram = nc.dram_tensor("wt", [F, 2 * OUT], f16, kind="ExternalInput")
    cam6_dram = nc.dram_tensor("cam6", [128, GROUP_ROWS], f16, kind="ExternalInput")
    cam2_dram = nc.dram_tensor("cam2", [128, 42], f16, kind="ExternalInput")
    bias_dram = nc.dram_tensor("biasr", [QUAD, OUT], f16, kind="ExternalInput")
    ident_dram = nc.dram_tensor("ident", [128, 128], f16, kind="ExternalInput")
    y_dram = nc.dram_tensor("y", [ROWS_PER_CORE, OUT], f16, kind="ExternalOutput")

    with tile.TileContext(nc) as tc:
        with (
            tc.tile_pool(name="consts", bufs=1) as cpool,
            tc.tile_pool(name="xb", bufs=2) as xbpool,
            tc.tile_pool(name="xrm", bufs=2) as xrmpool,
            tc.tile_pool(name="xt", bufs=2) as xtpool,
            tc.tile_pool(name="tpp", bufs=2, space=bass.MemorySpace.PSUM) as tpppool,
            tc.tile_pool(name="pp1", bufs=1, space=bass.MemorySpace.PSUM) as pp1pool,
            tc.tile_pool(name="py", bufs=2, space=bass.MemorySpace.PSUM) as pypool,
            tc.tile_pool(name="e", bufs=3) as epool,
            tc.tile_pool(name="e1", bufs=3) as e1pool,
            tc.tile_pool(name="y", bufs=2) as ypool,
        ):
            wt0_sb = cpool.tile([128, 2 * OUT], f16, tag="wt0")
            wt1_sb = cpool.tile([128, 2 * OUT], f16, tag="wt1")
            cam6_sb = cpool.tile([128, GROUP_ROWS], f16, tag="cam6")
            cam2_sb = cpool.tile([128, 42], f16, tag="cam2")
            ident_sb = cpool.tile([128, 128], f16, tag="ident")
            nc.sync.dma_start(wt0_sb[:, :], wt_dram[0:128, :])
            nc.sync.dma_start(wt1_sb[:, :], wt_dram[128:256, :])
            nc.sync.dma_start(cam6_sb[:, :], cam6_dram[:, :])
            nc.sync.dma_start(cam2_sb[:, :], cam2_dram[:, :])
            nc.sync.dma_start(ident_sb[:, :], ident_dram[:, :])
            # Rotating cam-matmul rhs tiles; bias row (partition GROUP_ROWS /
            # 42 for the tail tile) is written once here and never again.
            p1rot = [cpool.tile([128, QUAD, OUT], f16, tag=f"p1rot{i}",
                                name=f"p1rot{i}")
                     for i in range(4)]
            p1tail = cpool.tile([128, QUAD, OUT], f16, tag="p1tail")
            for t in p1rot:
                nc.sync.dma_start(t[GROUP_ROWS : GROUP_ROWS + 1, :, :],
                                  bias_dram[:, :])
            nc.sync.dma_start(p1tail[42:43, 0:1, :], bias_dram[0:1, :])

            consts = dict(wt0=wt0_sb, wt1=wt1_sb, cam6=cam6_sb, cam2=cam2_sb,
                          ident=ident_sb, p1rot=p1rot, p1tail=p1tail)
            pools = dict(tpp=tpppool, pp1=pp1pool, py=pypool, e=epool,
                         e1=e1pool, y=ypool)

            # Super-group descriptors: (r0, nxb, npe, n_full_groups, has_tail)
            n_full_sg = ROWS_PER_CORE // SG_ROWS  # 5
            sgs = [(sg * SG_ROWS, XBAR_CHUNKS, PE_CHUNKS, SG_ROWS // GROUP_ROWS,
                    False) for sg in range(n_full_sg)]
            sgs.append((n_full_sg * SG_ROWS, TAIL_XBAR_CHUNKS, TAIL_PE_CHUNKS,
                        (TAIL_SG_ROWS - 42) // GROUP_ROWS, True))

            def ld_tiles():
                return (
                    xbpool.tile([128, XBAR_CHUNKS, 128], f16, tag="xb0",
                                name="xb0"),
                    xbpool.tile([128, XBAR_CHUNKS, 128], f16, tag="xb1",
                                name="xb1"),
                    xrmpool.tile([128, PE_CHUNKS, 256], f16, tag="xrm",
                                 name="xrm"),
                )

            def xt_tiles():
                return (xtpool.tile([128, SG_ROWS], f16, tag="xt0", name="xt0"),
                        xtpool.tile([128, SG_ROWS], f16, tag="xt1", name="xt1"))

            # Prologue: loads + all transposes for SG0.
            lt = ld_tiles()
            _emit_loads(nc, lt, x_dram, sgs[0][0], sgs[0][1], sgs[0][2])
            xt = xt_tiles()
            for th in _make_transpose_thunks(nc, pools, consts, lt, xt,
                                             sgs[0][1], sgs[0][2]):
                th()

            flush = {"ysb": None}
            pending = collections.deque()
            qidx = 0
            for si, (r0, nxb, npe, n_full, has_tail) in enumerate(sgs):
                # Issue next SG's loads and build its transpose thunks.
                nxt_thunks = []
                if si + 1 < len(sgs):
                    nr0, nnxb, nnpe, _, _ = sgs[si + 1]
                    nlt = ld_tiles()
                    _emit_loads(nc, nlt, x_dram, nr0, nnxb, nnpe)
                    nxt = xt_tiles()
                    nxt_thunks = _make_transpose_thunks(
                        nc, pools, consts, nlt, nxt, nnxb, nnpe)

                quads = [(t0, min(QUAD, n_full - t0))
                         for t0 in range(0, n_full, QUAD)]
                if has_tail:
                    quads.append((n_full, -1))

                # Drain next-SG transpose thunks spread over the middle of
                # this SG's quad loop (loads have landed by then; xbar first
                # so the next SG's first groups are ready earliest).
                n_q = len(quads)
                sched = []
                if nxt_thunks:
                    d0 = min(4, max(0, n_q - len(nxt_thunks)))
                    span = max(1, n_q - d0)
                    sched = [d0 + (k * span) // len(nxt_thunks)
                             for k in range(len(nxt_thunks))]
                for qi, (t0, tn) in enumerate(quads):
                    while sched and sched[0] <= qi:
                        sched.pop(0)
                        nxt_thunks.pop(0)()
                    is_tail = tn == -1
                    st = dict(
                        q0=t0, qn=1 if is_tail else tn,
                        mrows=42 if is_tail else GROUP_ROWS,
                        is_tail=is_tail, qidx=qidx, r0=r0, y_dram=y_dram,
                    )
                    _emit_front(nc, pools, consts, xt, st)
                    if pending:
                        _emit_back(nc, pools, consts, pending.popleft(), flush)
                    pending.append(st)
                    qidx += 1
                for th in nxt_thunks:
                    th()
                if si + 1 < len(sgs):
                    xt = nxt

            while pending:
                _emit_back(nc, pools, consts, pending.popleft(), flush)

    nc.compile()
    return nc


_NC_CACHE = None


def _host_constants(cam, W, b):
    W = np.asarray(W, np.float32)
    cam = np.asarray(cam, np.float32)
    b = np.asarray(b, np.float32)
    # rhs of matmuls: [f, o2] with o2<256 -> W1.T, o2>=256 -> W2.T
    wt = np.concatenate([W[:, :F].T, W[:, F:].T], axis=1).astype(np.float16)
    # Block-diagonal cam.T (6 batches) + ones row for the bias term.
    cam6 = np.zeros((128, GROUP_ROWS), np.float32)
    for bb in range(GROUP_BATCHES):
        cam6[bb * N : (bb + 1) * N, bb * N : (bb + 1) * N] = cam.T
    cam6[GROUP_ROWS, :] = 1.0
    cam2 = np.zeros((128, 42), np.float32)
    for bb in range(2):
        cam2[bb * N : (bb + 1) * N, bb * N : (bb + 1) * N] = cam.T
    cam2[42, :] = 1.0
    biasr = np.tile(b.reshape(1, OUT), (QUAD, 1))
    ident = np.eye(128, dtype=np.float16)
    return (wt, cam6.astype(np.float16), cam2.astype(np.float16),
            biasr.astype(np.float16), ident)


def make_in_maps(x, cam, W, b):
    x16 = np.ascontiguousarray(np.asarray(x)).astype(np.float16)
    assert x16.shape == (N_CORES * ROWS_PER_CORE, F)
    wt, cam6, cam2, biasr, ident = _host_constants(cam, W, b)
    in_maps = []
    for i in range(N_CORES):
        in_maps.append({
            "xs": x16[i * ROWS_PER_CORE : (i + 1) * ROWS_PER_CORE, :],
            "wt": wt, "cam6": cam6, "cam2": cam2, "biasr": biasr,
            "ident": ident,
        })
    return in_maps


def kernel(x, cam, W, b, n_joints):
    global _NC_CACHE
    if _NC_CACHE is None:
        _NC_CACHE = _build_nc()
    nc = _NC_CACHE
    in_maps = make_in_maps(x, cam, W, b)
    res = run_bass_kernel_spmd(nc, in_maps, core_ids=list(range(N_CORES)))
    y = np.concatenate([res.results[i]["y"] for i in range(N_CORES)], axis=0)
    return y.astype(np.float32)


# revision 13
# speedup vs baseline: 3.9541x; 1.0113x over previous
"""CAM graph layer (message passing + Linear + ELU) on 8 Trainium2 NeuronCores.

Reference computation (per batch b of N=21 joints, F=256 features):
    x_agg[b,i] = sum_j cam[i,j] * x[b,j]            (21x21 aggregation)
    y = ELU(concat([x_agg, x], -1) @ W.T + b)       (Linear 512->256)

Kernel restructure:
    P1 = x @ W1.T ; P2 = x @ W2.T          (W1/W2 = halves of W)
    y[b,i] = sum_j cam[i,j]*P1[b,j] + P2[b,i] + bias
    ELU(y) = min(relu(y), exp(y)-1)

v3 design (per core, 43008 rows):
  - x arrives fp16 (host cast; numerically identical to the old SWDGE
    cast-load) -> 22MB HBM reads; y stored fp16, host upcast -> 22MB writes.
  - x must become feature-major for the matmuls. Transpose split: first 31
    chunks/SG via xbar DMA transpose, last 32 via PE is_transpose matmuls
    into fp16 PSUM + DVE 2x copies, balancing DMA vs PE time.
  - QUAD=4 groups per PSUM tile to amortize the fixed SBUF/PSUM access
    latency of every epilogue instruction. PSUM (8 banks): P1 pool
    [128,4,256]f32 bufs=1 (2) + Y pool [128,4,256]f32 bufs=2 (4) + PE
    transpose staging [128,8,128]f16 bufs=2 (2).
  - Epilogue engines (gpsimd cannot read PSUM and its ALU is ~15x slower
    than DVE -- never give it bulk work): P1 cast-copy + exp on Act,
    (exp-1) 4x + combine (y max 0) min (exp-1) + transpose copies on DVE.
  - Software pipeline skew 1 quad: PE order per quad is
    [P1-matmuls(q), P2-matmuls(q), cam-matmul(q-1)], so the Act P1-copy(q)
    overlaps the P2 matmuls and the cam matmul of the previous quad.
"""

import collections

import numpy as np

import concourse.bass as bass
import concourse.bacc as bacc
import concourse.mybir as mybir
import concourse.tile as tile
from concourse.bass_utils import run_bass_kernel_spmd

N_CORES = 8
N = 21
F = 256
OUT = 256
ROWS_PER_CORE = 43008          # 2048 batches * 21 joints
GROUP_BATCHES = 6
GROUP_ROWS = GROUP_BATCHES * N      # 126
SG_ROWS = 8064                 # lcm(128, 126): 63 chunks, 64 groups
TAIL_SG_ROWS = 2688            # 43008 - 5*8064: 21 chunks, 21 groups + 42 rows
XBAR_CHUNKS = 31               # per full SG: chunks 0..30 via xbar transpose
PE_CHUNKS = 32                 # chunks 31..62 via PE transpose (4 fills of 8/half)
TAIL_XBAR_CHUNKS = 13
TAIL_PE_CHUNKS = 8
FILL = 8                       # PE transposes per psum fill / DVE copy
STORE_GROUPS = 8               # groups per output store DMA (1008 rows)
QUAD = 4                       # groups per PSUM tile; batches epilogue ops

f16 = mybir.dt.float16
f32 = mybir.dt.float32

_ALU = mybir.AluOpType
_ACT = mybir.ActivationFunctionType


def _emit_loads(nc, tiles, x_dram, r0, nxb, npe):
    """Issue the 4 HWDGE loads for one super-group on the Act queue (no
    waits, hardware desc-gen -- the SWDGE ring serializes ~16us/SG and the
    SP queue must stay free so each xbar transpose can start the moment its
    source load lands). xbar feature-half tiles (256B descs) load FIRST so
    the xbar can go early; PE row-major chunks (512B descs) follow. The SG
    layout is [PE chunks 0..npe-1][xbar chunks npe..npe+nxb-1]."""
    xb0, xb1, xrm = tiles
    rx = r0 + npe * 128
    nc.scalar.dma_start(
        xb0[:, 0:nxb, :],
        x_dram[rx : rx + nxb * 128, 0:128].rearrange("(c p) f -> p c f", p=128),
    )
    nc.scalar.dma_start(
        xb1[:, 0:nxb, :],
        x_dram[rx : rx + nxb * 128, 128:256].rearrange("(c p) f -> p c f", p=128),
    )
    half = (npe // 2) * 128
    nc.scalar.dma_start(
        xrm[:, 0 : npe // 2, :],
        x_dram[r0 : r0 + half, :].rearrange("(c p) f -> p c f", p=128),
    )
    nc.scalar.dma_start(
        xrm[:, npe // 2 : npe, :],
        x_dram[r0 + half : r0 + npe * 128, :].rearrange("(c p) f -> p c f", p=128),
    )


def _make_transpose_thunks(nc, pools, consts, ld_tiles, xt_tiles, nxb, npe):
    """Build the list of transpose thunks for one super-group: 2 xbar
    instrs + one (8 PE transposes + DVE copy) thunk per fill."""
    tpppool = pools["tpp"]
    ident_sb = consts["ident"]
    xb0, xb1, xrm = ld_tiles
    xt0, xt1 = xt_tiles
    thunks = []

    def xbar(xb, xt):
        def run():
            c0 = npe * 128
            nc.sync.dma_start_transpose(
                xt[:, c0 : c0 + nxb * 128].rearrange("p (c q) -> p c q", q=128),
                xb[:, 0:nxb, :],
            )
        return run

    n_fills = npe // FILL
    for fi in range(n_fills):
        for h, xt in ((0, xt0), (1, xt1)):
            def fill(fi=fi, h=h, xt=xt):
                tpp = tpppool.tile([128, FILL, 128], f16, tag="tpp", name="tpp")
                for i in range(FILL):
                    nc.tensor.transpose(
                        tpp[:, i, :],
                        xrm[:, fi * FILL + i, h * 128 : (h + 1) * 128],
                        ident_sb[:, :],
                    )
                c0 = fi * FILL * 128
                nc.vector.tensor_copy(
                    xt[:, c0 : c0 + FILL * 128].rearrange("p (c q) -> p c q", q=128),
                    tpp[:, 0:FILL, :],
                )
            thunks.append(fill)

    return thunks, [xbar(xb0, xt0), xbar(xb1, xt1)]


def _emit_front(nc, pools, consts, xt_tiles, st):
    """Front half of one quad: P1 matmuls + Act P1 cast-copy + P2 matmuls."""
    wt0_sb, wt1_sb = consts["wt0"], consts["wt1"]
    p1rot, p1tail = consts["p1rot"], consts["p1tail"]
    xt0, xt1 = xt_tiles
    q0, qn, mrows, is_tail, qidx = (
        st["q0"], st["qn"], st["mrows"], st["is_tail"], st["qidx"])

    pp1 = pools["pp1"].tile([128, QUAD, OUT], f32, tag="pp1", name="pp1")
    py = pools["py"].tile([128, QUAD, OUT], f32, tag="py", name="py")
    st["py"] = py
    # start=True resets the WHOLE 2KB PSUM bank (2 group slots), so only the
    # first matmul touching a bank may set it; stop on the bank's last.
    for qq in range(qn):
        gr0 = (q0 + qq) * GROUP_ROWS
        first = qq % 2 == 0
        last = qq % 2 == 1 or qq == qn - 1
        nc.tensor.matmul(
            pp1[0:mrows, qq, :], xt0[:, gr0 : gr0 + mrows], wt0_sb[:, 0:OUT],
            start=first, stop=False, skip_group_check=True,
        )
        nc.tensor.matmul(
            pp1[0:mrows, qq, :], xt1[:, gr0 : gr0 + mrows], wt1_sb[:, 0:OUT],
            start=False, stop=last, skip_group_check=True,
        )

    p1t = p1tail if is_tail else p1rot[qidx % len(p1rot)]
    st["p1t"] = p1t
    nc.scalar.copy(p1t[0:mrows, 0:qn, :], pp1[0:mrows, 0:qn, :])

    for qq in range(qn):
        gr0 = (q0 + qq) * GROUP_ROWS
        first = qq % 2 == 0
        last = qq % 2 == 1 or qq == qn - 1
        nc.tensor.matmul(
            py[0:mrows, qq, :], xt0[:, gr0 : gr0 + mrows], wt0_sb[:, OUT : 2 * OUT],
            start=first, stop=False, skip_group_check=True,
        )
        nc.tensor.matmul(
            py[0:mrows, qq, :], xt1[:, gr0 : gr0 + mrows], wt1_sb[:, OUT : 2 * OUT],
            start=False, stop=last, skip_group_check=True,
        )


def _emit_back(nc, pools, consts, st, flush):
    """Back half of one quad: cam matmul + ELU + (maybe) store flush.

    Stores are deferred by one back() call so the Act sequencer never
    reaches a store before its DVE combine has finished (head-of-line
    stall on the in-order queue)."""
    if flush.get("store") is not None:
        flush.pop("store")()
    epool, e1pool, ypool = pools["e"], pools["e1"], pools["y"]
    py, p1t = st["py"], st["p1t"]
    q0, qn, mrows, is_tail = st["q0"], st["qn"], st["mrows"], st["is_tail"]
    cam_sb = consts["cam2"] if is_tail else consts["cam6"]
    y_dram = st["y_dram"]

    # Matmul out must stay within one PSUM bank (512 fp32): <=2 groups/instr.
    for h0 in range(0, qn, 2):
        hn = min(2, qn - h0)
        nc.tensor.matmul(
            py[0:mrows, h0 : h0 + hn, :],
            cam_sb[0 : mrows + 1, 0:mrows],
            p1t[0 : mrows + 1, h0 : h0 + hn, :],
            start=False, stop=True, skip_group_check=True,
        )

    # ELU(y) = min(relu(y), exp(y)-1): exp on Act, -1 on DVE 4x, single
    # PSUM-reading combine on DVE.
    esb = epool.tile([128, QUAD, OUT], f16, tag="esb", name="esb")
    nc.scalar.activation(
        esb[0:mrows, 0:qn, :], py[0:mrows, 0:qn, :], _ACT.Exp
    )
    e1t = e1pool.tile([128, QUAD, OUT], f16, tag="e1", name="e1")
    nc.vector.tensor_scalar_sub(e1t[0:mrows, 0:qn, :], esb[0:mrows, 0:qn, :], 1.0)

    if flush["ysb"] is None:
        flush["ysb"] = ypool.tile([128, STORE_GROUPS, OUT], f16, tag="ysb",
                                  name="ysb")
        flush["g0"] = q0
        flush["r0"] = st["r0"]
    ysb = flush["ysb"]
    slot = q0 - flush["g0"]
    nc.vector.scalar_tensor_tensor(
        ysb[0:mrows, slot : slot + qn, :],
        py[0:mrows, 0:qn, :], 0.0, e1t[0:mrows, 0:qn, :],
        _ALU.max, _ALU.min,
    )
    if is_tail:
        rf0 = flush["r0"] + flush["g0"] * GROUP_ROWS
        rt0 = flush["r0"] + q0 * GROUP_ROWS

        def store(ysb=ysb, rf0=rf0, rt0=rt0, slot=slot):
            if slot > 0:
                nc.scalar.dma_start(
                    y_dram[rf0 : rf0 + slot * GROUP_ROWS, :].rearrange(
                        "(g p) f -> p g f", p=GROUP_ROWS
                    ),
                    ysb[0:GROUP_ROWS, 0:slot, :],
                )
            nc.scalar.dma_start(y_dram[rt0 : rt0 + 42, :], ysb[0:42, slot, :])

        flush["store"] = store
        flush["ysb"] = None
    elif slot + qn == STORE_GROUPS:
        rf0 = flush["r0"] + flush["g0"] * GROUP_ROWS

        def store(ysb=ysb, rf0=rf0, ng=slot + qn):
            nc.scalar.dma_start(
                y_dram[rf0 : rf0 + ng * GROUP_ROWS, :].rearrange(
                    "(g p) f -> p g f", p=GROUP_ROWS
                ),
                ysb[0:GROUP_ROWS, 0:ng, :],
            )

        flush["store"] = store
        flush["ysb"] = None


def _build_nc():
    nc = bacc.Bacc("TRN2", target_bir_lowering=False, debug=False,
                   num_devices=N_CORES)
    x_dram = nc.dram_tensor("xs", [ROWS_PER_CORE, F], f16, kind="ExternalInput")
    wt_dram = nc.dram_tensor("wt", [F, 2 * OUT], f16, kind="ExternalInput")
    cam6_dram = nc.dram_tensor("cam6", [128, GROUP_ROWS], f16, kind="ExternalInput")
    cam2_dram = nc.dram_tensor("cam2", [128, 42], f16, kind="ExternalInput")
    bias_dram = nc.dram_tensor("biasr", [QUAD, OUT], f16, kind="ExternalInput")
    ident_dram = nc.dram_tensor("ident", [128, 128], f16, kind="ExternalInput")
    y_dram = nc.dram_tensor("y", [ROWS_PER_CORE, OUT], f16, kind="ExternalOutput")

    with tile.TileContext(nc) as tc:
        with (
            tc.tile_pool(name="consts", bufs=1) as cpool,
            tc.tile_pool(name="xb", bufs=2) as xbpool,
            tc.tile_pool(name="xrm", bufs=2) as xrmpool,
            tc.tile_pool(name="xt", bufs=2) as xtpool,
            tc.tile_pool(name="tpp", bufs=2, space=bass.MemorySpace.PSUM) as tpppool,
            tc.tile_pool(name="pp1", bufs=1, space=bass.MemorySpace.PSUM) as pp1pool,
            tc.tile_pool(name="py", bufs=2, space=bass.MemorySpace.PSUM) as pypool,
            tc.tile_pool(name="e", bufs=3) as epool,
            tc.tile_pool(name="e1", bufs=3) as e1pool,
            tc.tile_pool(name="y", bufs=2) as ypool,
        ):
            wt0_sb = cpool.tile([128, 2 * OUT], f16, tag="wt0")
            wt1_sb = cpool.tile([128, 2 * OUT], f16, tag="wt1")
            cam6_sb = cpool.tile([128, GROUP_ROWS], f16, tag="cam6")
            cam2_sb = cpool.tile([128, 42], f16, tag="cam2")
            ident_sb = cpool.tile([128, 128], f16, tag="ident")
            nc.sync.dma_start(wt0_sb[:, :], wt_dram[0:128, :])
            nc.sync.dma_start(wt1_sb[:, :], wt_dram[128:256, :])
            nc.sync.dma_start(cam6_sb[:, :], cam6_dram[:, :])
            nc.sync.dma_start(cam2_sb[:, :], cam2_dram[:, :])
            nc.sync.dma_start(ident_sb[:, :], ident_dram[:, :])
            # Rotating cam-matmul rhs tiles; bias row (partition GROUP_ROWS /
            # 42 for the tail tile) is written once here and never again.
            p1rot = [cpool.tile([128, QUAD, OUT], f16, tag=f"p1rot{i}",
                                name=f"p1rot{i}")
                     for i in range(4)]
            p1tail = cpool.tile([128, QUAD, OUT], f16, tag="p1tail")
            for t in p1rot:
                nc.sync.dma_start(t[GROUP_ROWS : GROUP_ROWS + 1, :, :],
                                  bias_dram[:, :])
            nc.sync.dma_start(p1tail[42:43, 0:1, :], bias_dram[0:1, :])

            consts = dict(wt0=wt0_sb, wt1=wt1_sb, cam6=cam6_sb, cam2=cam2_sb,
                          ident=ident_sb, p1rot=p1rot, p1tail=p1tail)
            pools = dict(tpp=tpppool, pp1=pp1pool, py=pypool, e=epool,
                         e1=e1pool, y=ypool)

            # Super-group descriptors: (r0, nxb, npe, n_full_groups, has_tail)
            n_full_sg = ROWS_PER_CORE // SG_ROWS  # 5
            sgs = [(sg * SG_ROWS, XBAR_CHUNKS, PE_CHUNKS, SG_ROWS // GROUP_ROWS,
                    False) for sg in range(n_full_sg)]
            sgs.append((n_full_sg * SG_ROWS, TAIL_XBAR_CHUNKS, TAIL_PE_CHUNKS,
                        (TAIL_SG_ROWS - 42) // GROUP_ROWS, True))

            def ld_tiles():
                return (
                    xbpool.tile([128, XBAR_CHUNKS, 128], f16, tag="xb0",
                                name="xb0"),
                    xbpool.tile([128, XBAR_CHUNKS, 128], f16, tag="xb1",
                                name="xb1"),
                    xrmpool.tile([128, PE_CHUNKS, 256], f16, tag="xrm",
                                 name="xrm"),
                )

            def xt_tiles():
                return (xtpool.tile([128, SG_ROWS], f16, tag="xt0", name="xt0"),
                        xtpool.tile([128, SG_ROWS], f16, tag="xt1", name="xt1"))

            # Prologue: loads + all transposes for SG0.
            lt = ld_tiles()
            _emit_loads(nc, lt, x_dram, sgs[0][0], sgs[0][1], sgs[0][2])
            xt = xt_tiles()
            fills0, xbars0 = _make_transpose_thunks(nc, pools, consts, lt, xt,
                                                    sgs[0][1], sgs[0][2])
            for th in xbars0 + fills0:
                th()

            flush = {"ysb": None}
            pending = collections.deque()
            qidx = 0
            for si, (r0, nxb, npe, n_full, has_tail) in enumerate(sgs):
                # Issue next SG's loads and build its transpose thunks.
                nxt_thunks = []
                if si + 1 < len(sgs):
                    nr0, nnxb, nnpe, _, _ = sgs[si + 1]
                    nlt = ld_tiles()
                    _emit_loads(nc, nlt, x_dram, nr0, nnxb, nnpe)
                    nxt = xt_tiles()
                    nxt_thunks, nxt_xbars = _make_transpose_thunks(
                        nc, pools, consts, nlt, nxt, nnxb, nnpe)
                    # Issue xbar transposes now: the dedicated SP queue just
                    # waits on the xb loads and fires them immediately.
                    for th in nxt_xbars:
                        th()

                quads = [(t0, min(QUAD, n_full - t0))
                         for t0 in range(0, n_full, QUAD)]
                if has_tail:
                    quads.append((n_full, -1))

                # Drain next-SG transpose thunks spread over the middle of
                # this SG's quad loop (loads have landed by then; xbar first
                # so the next SG's first groups are ready earliest).
                n_q = len(quads)
                sched = []
                if nxt_thunks:
                    d0 = min(4, max(0, n_q - len(nxt_thunks)))
                    span = max(1, n_q - d0)
                    sched = [d0 + (k * span) // len(nxt_thunks)
                             for k in range(len(nxt_thunks))]
                for qi, (t0, tn) in enumerate(quads):
                    while sched and sched[0] <= qi:
                        sched.pop(0)
                        nxt_thunks.pop(0)()
                    is_tail = tn == -1
                    st = dict(
                        q0=t0, qn=1 if is_tail else tn,
                        mrows=42 if is_tail else GROUP_ROWS,
                        is_tail=is_tail, qidx=qidx, r0=r0, y_dram=y_dram,
                    )
                    _emit_front(nc, pools, consts, xt, st)
                    if pending:
                        _emit_back(nc, pools, consts, pending.popleft(), flush)
                    pending.append(st)
                    qidx += 1
                for th in nxt_thunks:
                    th()
                if si + 1 < len(sgs):
                    xt = nxt

            while pending:
                _emit_back(nc, pools, consts, pending.popleft(), flush)
            if flush.get("store") is not None:
                flush.pop("store")()

    nc.compile()
    return nc


_NC_CACHE = None


def _host_constants(cam, W, b):
    W = np.asarray(W, np.float32)
    cam = np.asarray(cam, np.float32)
    b = np.asarray(b, np.float32)
    # rhs of matmuls: [f, o2] with o2<256 -> W1.T, o2>=256 -> W2.T
    wt = np.concatenate([W[:, :F].T, W[:, F:].T], axis=1).astype(np.float16)
    # Block-diagonal cam.T (6 batches) + ones row for the bias term.
    cam6 = np.zeros((128, GROUP_ROWS), np.float32)
    for bb in range(GROUP_BATCHES):
        cam6[bb * N : (bb + 1) * N, bb * N : (bb + 1) * N] = cam.T
    cam6[GROUP_ROWS, :] = 1.0
    cam2 = np.zeros((128, 42), np.float32)
    for bb in range(2):
        cam2[bb * N : (bb + 1) * N, bb * N : (bb + 1) * N] = cam.T
    cam2[42, :] = 1.0
    biasr = np.tile(b.reshape(1, OUT), (QUAD, 1))
    ident = np.eye(128, dtype=np.float16)
    return (wt, cam6.astype(np.float16), cam2.astype(np.float16),
            biasr.astype(np.float16), ident)


def make_in_maps(x, cam, W, b):
    x16 = np.ascontiguousarray(np.asarray(x)).astype(np.float16)
    assert x16.shape == (N_CORES * ROWS_PER_CORE, F)
    wt, cam6, cam2, biasr, ident = _host_constants(cam, W, b)
    in_maps = []
    for i in range(N_CORES):
        in_maps.append({
            "xs": x16[i * ROWS_PER_CORE : (i + 1) * ROWS_PER_CORE, :],
            "wt": wt, "cam6": cam6, "cam2": cam2, "biasr": biasr,
            "ident": ident,
        })
    return in_maps


def kernel(x, cam, W, b, n_joints):
    global _NC_CACHE
    if _NC_CACHE is None:
        _NC_CACHE = _build_nc()
    nc = _NC_CACHE
    in_maps = make_in_maps(x, cam, W, b)
    res = run_bass_kernel_spmd(nc, in_maps, core_ids=list(range(N_CORES)))
    y = np.concatenate([res.results[i]["y"] for i in range(N_CORES)], axis=0)
    return y.astype(np.float32)
